# revision 10
# baseline (speedup 1.0000x reference)
"""EvaAttention Trainium2 Bass kernel (bf16, pair-packed attention).

Strategy: data-parallel over batch across 8 cores, 4 batches per core.
All weights replicated; no collectives.

Per-core dataflow (transposed layouts keep every contraction dim on SBUF
partitions with no on-chip input transposes):
  xT [768, 4*577+2] resident in SBUF (host-pretransposed x, bf16)
  A:  qkT[o, t] = wT[:, :1536].T @ xT     (q,k head-transposed, o-major)
  B:  v[t, o]   = xT.T @ wT[:, 1536:]     (v token-major)
  R:  partial RoPE via constant permutation matmul (rot) + DVE mul/add.
      Host pre-permutes q/k head d-dims to [evens, odds] (S is invariant
      under a shared per-head d-permutation) making rot() a 32-row swap.
  S:  S.T[k, q] per head on PE (contraction d=64).  The two heads of an
      o-tile live on partition halves 0:64 / 64:128, so their S matmuls
      carry tile_position (0,0)/(64,0) and run CONCURRENTLY in the PE
      array when issued back-to-back.
  E:  exp(S.T/8) on ACT into bf16 (no max subtraction: |S/8| <~ 6 here).
      512-wide ops per k-chunk + one merged op for the 66-wide tails.
  PV: O.T[d, q] = [V | 1 | 1].T @ E accumulated over k-chunks into a
      2-bank [128,578] PSUM tile; row 64 = softmax denominators.
  renorm: 1/denom (DVE, bf16 row) -> broadcast to 64 partitions with a
      K=1 ones matmul on PE (no DMA round-trip) -> DVE mul into oT.
      Odd heads go via an SBUF staging tile + 64-partition-shift DMA.
  P:  y[t, o] = oT.T @ pwT, DMA'd to DRAM straight out of PSUM when
      proj_b == 0 (biases are detected host-side; nonzero biases take a
      DVE add path).
"""

import os
import numpy as np

import concourse.bacc as bacc
import concourse.bass as bass_mod
import concourse.tile as tile
import concourse.mybir as mybir
from concourse.bass_utils import run_bass_kernel_spmd

B, N, C = 32, 577, 768
H, D = 12, 64
NPT = 1
N_CORES = 8
NB = B // N_CORES          # batches per core
TT = NB * N                # tokens per core
N2 = N + 1                 # padded token count (578)
SCALE = D ** -0.5

K_REP = int(os.environ.get("BASS_K_REP", "1"))

_f32 = mybir.dt.float32
_BF = mybir.dt.bfloat16
_NP_BF = mybir.dt.np(_BF)


def preprocess(x, rope, qkv_w, q_bias, v_bias, proj_w, proj_b):
    """Host-side prep: transposes, head-d permutation, rope tables."""
    perm = np.concatenate([np.arange(0, D, 2), np.arange(1, D, 2)])
    rows = np.arange(3 * C)
    for region in (0, 1):  # q, k head-d reorder; v untouched
        for h in range(H):
            base = region * C + h * D
            rows[base : base + D] = base + perm
    qkv_w_p = np.asarray(qkv_w, np.float32)[rows]
    qkvb_flat = np.concatenate(
        [np.asarray(q_bias, np.float32), np.zeros(C, np.float32),
         np.asarray(v_bias, np.float32)])[rows]
    # [128, 18] column-per-o-tile layout for per-partition ACT bias
    qkvb = np.ascontiguousarray(qkvb_flat.reshape(18, 128).T)

    wT = np.ascontiguousarray(qkv_w_p.T).astype(_NP_BF)          # [768, 2304]
    pwT = np.ascontiguousarray(np.asarray(proj_w, np.float32).T).astype(_NP_BF)

    rope = np.asarray(rope, np.float32)
    sinT = np.ascontiguousarray(rope[:, :D].T[perm])             # [64, 576]
    cosT = np.ascontiguousarray(rope[:, D:].T[perm])
    cosT2 = np.concatenate([cosT, cosT], 0).astype(_NP_BF)       # [128, 576]
    sinT2 = np.concatenate([sinT, sinT], 0).astype(_NP_BF)

    rotm = np.zeros((128, 128), np.float32)
    for blk in range(2):
        o = blk * 64
        for j in range(32):
            rotm[o + 32 + j, o + j] = -1.0   # out[j]    = -rhs[32+j]
            rotm[o + j, o + 32 + j] = 1.0    # out[32+j] = +rhs[j]
    rotm = rotm.astype(_NP_BF)

    x = np.asarray(x, np.float32)
    xTs = []
    for core in range(N_CORES):
        xc = x[core * NB : (core + 1) * NB]                      # [NB, 577, 768]
        xTs.append(np.ascontiguousarray(
            xc.transpose(2, 0, 1).reshape(C, TT)).astype(_NP_BF))

    vb = np.asarray(v_bias, np.float32)
    pb = np.asarray(proj_b, np.float32)
    zero_bias = (not qkvb_flat.any()) and (not pb.any())
    return xTs, dict(wT=wT, pwT=pwT, qkvb=qkvb, vb=vb, pb=pb,
                     cosT2=cosT2, sinT2=sinT2, rotm=rotm), zero_bias


def build(zero_bias=True, k_rep=K_REP):
    nc = bacc.Bacc("TRN2", target_bir_lowering=False, debug=False,
                   num_devices=N_CORES)

    d_xT = nc.dram_tensor("xT", [C, TT], _BF, kind="ExternalInput").ap()
    d_wT = nc.dram_tensor("wT", [C, 3 * C], _BF, kind="ExternalInput").ap()
    d_pwT = nc.dram_tensor("pwT", [C, C], _BF, kind="ExternalInput").ap()
    d_qkvb = nc.dram_tensor("qkvb", [128, 18], _f32, kind="ExternalInput").ap()
    d_vb = nc.dram_tensor("vb", [C], _f32, kind="ExternalInput").ap()
    d_pb = nc.dram_tensor("pb", [C], _f32, kind="ExternalInput").ap()
    d_cos = nc.dram_tensor("cosT2", [128, N - 1], _BF, kind="ExternalInput").ap()
    d_sin = nc.dram_tensor("sinT2", [128, N - 1], _BF, kind="ExternalInput").ap()
    d_rotm = nc.dram_tensor("rotm", [128, 128], _BF, kind="ExternalInput").ap()
    d_out = nc.dram_tensor("out", [TT, C], _f32, kind="ExternalOutput").ap()

    Id = mybir.ActivationFunctionType.Identity
    Exp = mybir.ActivationFunctionType.Exp

    def _row_bc(ap, parts):
        return bass_mod.AP(tensor=ap.tensor, offset=ap.offset,
                           ap=[[0, parts]] + list(ap.ap))

    with tile.TileContext(nc) as tc:
        with tc.tile_pool(name="main", bufs=1) as pool, \
             tc.tile_pool(name="ps", bufs=1, space="PSUM") as pspool:

            # ---- resident constants ----
            wT_sb = pool.tile([128, 6, 3 * C], _BF, tag="wT")
            pwT_sb = pool.tile([128, 6, C], _BF, tag="pwT")
            for c in range(6):
                nc.sync.dma_start(out=wT_sb[:, c, :], in_=d_wT[c * 128:(c + 1) * 128, :])
                nc.sync.dma_start(out=pwT_sb[:, c, :], in_=d_pwT[c * 128:(c + 1) * 128, :])
            cos_sb = pool.tile([128, N - 1], _BF, tag="cos")
            nc.sync.dma_start(out=cos_sb, in_=d_cos)
            sin_sb = pool.tile([128, N - 1], _BF, tag="sin")
            nc.sync.dma_start(out=sin_sb, in_=d_sin)
            rotm_sb = pool.tile([128, 128], _BF, tag="rotm")
            nc.sync.dma_start(out=rotm_sb, in_=d_rotm)
            # ones row at partition 64 for the K=1 denominator broadcast
            ones_sb = pool.tile([128, 64], _BF, tag="ones")
            nc.vector.memset(ones_sb[64:65, :], 1.0)

            if not zero_bias:
                qkvb_sb = pool.tile([128, 18], _f32, tag="qkvb")
                nc.sync.dma_start(out=qkvb_sb, in_=d_qkvb)
                vb_bc = pool.tile([128, C], _f32, tag="vb")
                nc.sync.dma_start(out=vb_bc, in_=_row_bc(d_vb, 128))
                pb_bc = pool.tile([128, C], _f32, tag="pb")
                nc.sync.dma_start(out=pb_bc, in_=_row_bc(d_pb, 128))

            # resident xT (whole core's tokens; col TT..TT+1 zero pad)
            xT_sb = pool.tile([128, 6, TT + 2], _BF, tag="xT")
            nc.vector.memset(xT_sb[:, :, TT:TT + 2], 0.0)
            for c in range(6):
                nc.sync.dma_start(out=xT_sb[:, c, 0:TT],
                                  in_=d_xT[c * 128:(c + 1) * 128, :])

            def body():
                for b in range(NB):
                    t0 = b * N
                    xs = lambda c: xT_sb[:, c, t0:t0 + N2]

                    qk_sb = pool.tile([128, 12, N2], _BF, tag="qk", bufs=2)
                    v_sb = pool.tile([128, 5, H, D + 2], _BF, tag="vsb", bufs=2)
                    oT_sb = pool.tile([128, 6, N2], _BF, tag="oT", bufs=2)
                    nc.vector.memset(qk_sb[:, :, N:N2], 0.0)
                    nc.vector.memset(oT_sb[:, :, N:N2], 0.0)
                    # ones columns everywhere, then zero the pad rows
                    # (65:128) of the last k-chunk, then restore row 64's
                    # ones (row 64 = token 576's V, rewritten by phase B;
                    # rows 65: must be fully zero to drop out of PV).
                    nc.vector.memset(v_sb[:, :, :, D:D + 2], 1.0)
                    nc.vector.memset(v_sb[64:128, 4, :, :], 0.0)
                    nc.vector.memset(v_sb[64:65, 4, :, D:D + 2], 1.0)

                    # ---- A: q,k (o-major) ----
                    for ot in range(12):
                        psA = pspool.tile([128, 512], _f32, tag="w512", bufs=2)
                        psA2 = pspool.tile([128, 66], _f32, tag="w66", bufs=2)
                        for c in range(6):
                            lhsT = wT_sb[:, c, ot * 128:(ot + 1) * 128]
                            nc.tensor.matmul(psA, lhsT, xs(c)[:, 0:512],
                                             start=c == 0, stop=c == 5)
                            nc.tensor.matmul(psA2, lhsT, xs(c)[:, 512:N2],
                                             start=c == 0, stop=c == 5)
                        if zero_bias:
                            nc.scalar.activation(qk_sb[:, ot, 0:512], psA, Id)
                            nc.scalar.activation(qk_sb[:, ot, 512:N2],
                                                 psA2, Id)
                        else:
                            bias = qkvb_sb[:, ot:ot + 1]
                            nc.scalar.activation(qk_sb[:, ot, 0:512], psA, Id,
                                                 bias=bias)
                            nc.scalar.activation(qk_sb[:, ot, 512:N],
                                                 psA2[:, 0:65], Id, bias=bias)

                    # ---- B: v (t-major) ----
                    for tt in range(5):
                        P = min(128, N - tt * 128)   # true rows (65 on last)
                        PM = P + (P % 2)
                        psB = pspool.tile([128, 512], _f32, tag="w512", bufs=2)
                        psB2 = pspool.tile([128, 256], _f32, tag="w66", bufs=2)
                        for c in range(6):
                            lhsT = xs(c)[:, tt * 128:tt * 128 + PM]
                            nc.tensor.matmul(psB[:PM], lhsT,
                                             wT_sb[:, c, 1536:2048],
                                             start=c == 0, stop=c == 5)
                            nc.tensor.matmul(psB2[:PM], lhsT,
                                             wT_sb[:, c, 2048:2304],
                                             start=c == 0, stop=c == 5)
                        if zero_bias:
                            nc.vector.tensor_copy(
                                v_sb[0:P, tt, 0:8, 0:D],
                                psB[:P].rearrange("p (h d) -> p h d", d=D))
                            nc.vector.tensor_copy(
                                v_sb[0:P, tt, 8:12, 0:D],
                                psB2[:P].rearrange("p (h d) -> p h d", d=D))
                        else:
                            nc.vector.tensor_add(
                                v_sb[0:P, tt, 0:8, 0:D],
                                psB[:P].rearrange("p (h d) -> p h d", d=D),
                                vb_bc[0:P, 0:512].rearrange("p (h d) -> p h d", d=D))
                            nc.vector.tensor_add(
                                v_sb[0:P, tt, 8:12, 0:D],
                                psB2[:P].rearrange("p (h d) -> p h d", d=D),
                                vb_bc[0:P, 512:768].rearrange("p (h d) -> p h d", d=D))

                    # ---- R: rope on q,k ----
                    for ot in range(12):
                        psR = pspool.tile([128, 512], _f32, tag="w512", bufs=2)
                        psR2 = pspool.tile([128, 64], _f32, tag="w66", bufs=2)
                        nc.tensor.matmul(psR, rotm_sb, qk_sb[:, ot, 1:513],
                                         start=True, stop=True)
                        nc.tensor.matmul(psR2, rotm_sb,
                                         qk_sb[:, ot, 513:N],
                                         start=True, stop=True)
                        tmp = pool.tile([128, N - 1], _BF, tag="rtmp", bufs=2)
                        nc.vector.tensor_mul(tmp[:, 0:512], psR, sin_sb[:, 0:512])
                        nc.vector.tensor_mul(tmp[:, 512:N - 1], psR2,
                                             sin_sb[:, 512:N - 1])
                        nc.vector.tensor_mul(qk_sb[:, ot, 1:N], qk_sb[:, ot, 1:N],
                                             cos_sb)
                        nc.vector.tensor_add(qk_sb[:, ot, 1:N], qk_sb[:, ot, 1:N],
                                             tmp)

                    # ---- attention per head pair ----
                    for ot in range(6):
                        E = [pool.tile([128, 5, N2], _BF, tag="E", bufs=4,
                                       name=f"E{hb}")
                             for hb in (0, 64)]
                        psS66 = [pspool.tile([128, 5, 66], _f32, tag="w66",
                                             bufs=2, name=f"psS66_{hb}")
                                 for hb in (0, 64)]
                        # S + exp, pair-concurrent on PE
                        for kc in range(5):
                            KP = min(128, N2 - kc * 128)  # 128 or 66 padded
                            psS = [pspool.tile([128, 512], _f32, tag="w512",
                                               bufs=2, name=f"psS_{hb}")
                                   for hb in (0, 64)]
                            for i, hb in enumerate((0, 64)):
                                kk = qk_sb[hb:hb + 64, 6 + ot,
                                           kc * 128:kc * 128 + KP]
                                qq1 = qk_sb[hb:hb + 64, ot, 0:512]
                                nc.tensor.matmul(psS[i][:KP], kk, qq1,
                                                 start=True, stop=True)
                            for i, hb in enumerate((0, 64)):
                                kk = qk_sb[hb:hb + 64, 6 + ot,
                                           kc * 128:kc * 128 + KP]
                                qq2 = qk_sb[hb:hb + 64, ot, 512:N2]
                                nc.tensor.matmul(psS66[i][:KP, kc, :], kk, qq2,
                                                 start=True, stop=True)
                            for i in range(2):
                                nc.scalar.activation(E[i][0:KP, kc, 0:512],
                                                     psS[i][:KP], Exp,
                                                     scale=SCALE)
                        # merged tail exps (rows beyond KP of kc=4 are junk
                        # PSUM; exp only the valid rows)
                        for i in range(2):
                            nc.scalar.activation(E[i][:, 0:4, 512:N2],
                                                 psS66[i][:, 0:4, :], Exp,
                                                 scale=SCALE)
                            nc.scalar.activation(E[i][0:66, 4, 512:N2],
                                                 psS66[i][0:66, 4, :], Exp,
                                                 scale=SCALE)

                        # PV accumulation: psO [128, 578] spans 2 banks
                        psO = [pspool.tile([128, N2], _f32, tag="w578",
                                           bufs=2, name=f"psO_{hb}")
                               for hb in (0, 64)]
                        for kc in range(5):
                            KP = min(128, N2 - kc * 128)
                            for i, hb in enumerate((0, 64)):
                                vv = v_sb[0:KP, kc, 2 * ot + i, :]
                                nc.tensor.matmul(psO[i][:D + 2, 0:512], vv,
                                                 E[i][0:KP, kc, 0:512],
                                                 start=kc == 0, stop=kc == 4)
                                nc.tensor.matmul(psO[i][:D + 2, 512:N2], vv,
                                                 E[i][0:KP, kc, 512:N2],
                                                 start=kc == 0, stop=kc == 4)

                        # renorm: reciprocal -> PE broadcast -> DVE mul
                        for i, hb in enumerate((0, 64)):
                            r2 = pool.tile([66, N2], _BF, tag="r2", bufs=4)
                            with nc.allow_low_precision(
                                    reason="bf16 softmax denom reciprocal"):
                                nc.vector.reciprocal(r2[64:65, :],
                                                     psO[i][64:65, 0:N2])
                            psRB = pspool.tile([128, 512], _f32, tag="w512",
                                               bufs=2)
                            psRB2 = pspool.tile([128, 66], _f32, tag="w66",
                                                bufs=2)
                            nc.tensor.matmul(psRB[0:64], ones_sb[64:65, :],
                                             r2[64:65, 0:512],
                                             start=True, stop=True)
                            nc.tensor.matmul(psRB2[0:64],
                                             ones_sb[64:65, :],
                                             r2[64:65, 512:N2],
                                             start=True, stop=True)
                            # DVE can read only one PSUM operand per op, so
                            # stage the broadcast rows in SBUF (ACT + DVE).
                            rb = pool.tile([64, N2], _BF, tag="rb", bufs=4)
                            nc.scalar.activation(rb[:, 0:512], psRB[0:64], Id)
                            nc.vector.tensor_copy(rb[:, 512:N2], psRB2[0:64])
                            if hb == 0:
                                nc.vector.tensor_mul(oT_sb[0:64, ot, 0:512],
                                                     psO[i][0:64, 0:512],
                                                     rb[:, 0:512])
                                nc.vector.tensor_mul(oT_sb[0:64, ot, 512:N2],
                                                     psO[i][0:64, 512:N2],
                                                     rb[:, 512:N2])
                            else:
                                otmp = pool.tile([64, N2], _BF, tag="otmp",
                                                 bufs=3)
                                nc.vector.tensor_mul(otmp[:, 0:512],
                                                     psO[i][0:64, 0:512],
                                                     rb[:, 0:512])
                                nc.vector.tensor_mul(otmp[:, 512:N2],
                                                     psO[i][0:64, 512:N2],
                                                     rb[:, 512:N2])
                                nc.sync.dma_start(out=oT_sb[64:128, ot, 0:N2],
                                                  in_=otmp)

                    # ---- P: output projection ----
                    for tt in range(5):
                        P = min(128, N - tt * 128)
                        PM = P + (P % 2)
                        psP = pspool.tile([128, 512], _f32, tag="w512", bufs=2)
                        psP2 = pspool.tile([128, 256], _f32, tag="w66", bufs=2)
                        for c in range(6):
                            lhsT = oT_sb[:, c, tt * 128:tt * 128 + PM]
                            nc.tensor.matmul(psP[:PM], lhsT, pwT_sb[:, c, 0:512],
                                             start=c == 0, stop=c == 5)
                            nc.tensor.matmul(psP2[:PM], lhsT,
                                             pwT_sb[:, c, 512:C],
                                             start=c == 0, stop=c == 5)
                        yt = pool.tile([128, C], _f32, tag="y", bufs=2)
                        if zero_bias:
                            nc.vector.tensor_copy(yt[:P, 0:512], psP[:P])
                            nc.vector.tensor_copy(yt[:P, 512:C], psP2[:P])
                        else:
                            nc.vector.tensor_add(yt[:P, 0:512], psP[:P],
                                                 pb_bc[:P, 0:512])
                            nc.vector.tensor_add(yt[:P, 512:C], psP2[:P],
                                                 pb_bc[:P, 512:C])
                        nc.sync.dma_start(
                            out=d_out[t0 + tt * 128:t0 + tt * 128 + P, :],
                            in_=yt[:P, :])

            if k_rep > 1:
                with tc.For_i(0, k_rep, 1):
                    body()
            else:
                body()

    nc.compile()
    return nc


_CACHE = {}


def _get_nc(zero_bias=True, k_rep=K_REP):
    key = (zero_bias, k_rep)
    if key not in _CACHE:
        _CACHE[key] = build(zero_bias, k_rep)
    return _CACHE[key]


def kernel(**inputs) -> np.ndarray:
    xTs, pre, zero_bias = preprocess(**inputs)
    nc = _get_nc(zero_bias)
    shared = {k: pre[k] for k in
              ("wT", "pwT", "qkvb", "vb", "pb", "cosT2", "sinT2", "rotm")}
    in_maps = [dict(shared, xT=xTs[core]) for core in range(N_CORES)]
    res = run_bass_kernel_spmd(nc, in_maps, list(range(N_CORES)))
    out = np.concatenate(
        [res.results[c]["out"].reshape(NB, N, C) for c in range(N_CORES)], axis=0)
    return out


# revision 15
# speedup vs baseline: 1.6089x; 1.6089x over previous
"""EvaAttention Trainium2 Bass kernel (bf16, pair-packed attention).

Strategy: data-parallel over batch across 8 cores, 4 batches per core.
All weights replicated; no collectives.

Per-core dataflow (transposed layouts keep every contraction dim on SBUF
partitions with no on-chip input transposes):
  xT [768, 4*577+2] resident in SBUF (host-pretransposed x, bf16)
  A:  qkT[o, t] = wT[:, :1536].T @ xT     (q,k head-transposed, o-major)
  B:  v[t, o]   = xT.T @ wT[:, 1536:]     (v token-major)
  R:  partial RoPE via constant permutation matmul (rot) + DVE mul/add.
      Host pre-permutes q/k head d-dims to [evens, odds] (S is invariant
      under a shared per-head d-permutation) making rot() a 32-row swap.
  S:  S.T[k, q] per head on PE (contraction d=64).  The two heads of an
      o-tile live on partition halves 0:64 / 64:128, so their S matmuls
      carry tile_position (0,0)/(64,0) and run CONCURRENTLY in the PE
      array when issued back-to-back.
  E:  exp(S.T/8) on ACT into bf16 (no max subtraction: |S/8| <~ 6 here).
      512-wide ops per k-chunk + one merged op for the 66-wide tails.
  PV: O.T[d, q] = [V | 1 | 1].T @ E accumulated over k-chunks into a
      2-bank [128,578] PSUM tile; row 64 = softmax denominators.
  renorm: 1/denom (DVE, bf16 row) -> broadcast to 64 partitions with a
      K=1 ones matmul on PE (no DMA round-trip) -> DVE mul into oT.
      Odd heads go via an SBUF staging tile + 64-partition-shift DMA.
  P:  y[t, o] = oT.T @ pwT, DMA'd to DRAM straight out of PSUM when
      proj_b == 0 (biases are detected host-side; nonzero biases take a
      DVE add path).
"""

import os
import numpy as np

import concourse.bacc as bacc
import concourse.bass as bass_mod
import concourse.tile as tile
import concourse.mybir as mybir
from concourse.bass_utils import run_bass_kernel_spmd

B, N, C = 32, 577, 768
H, D = 12, 64
NPT = 1
N_CORES = 8
NB = B // N_CORES          # batches per core
TT = NB * N                # tokens per core
N2 = N + 1                 # padded token count (578)
SCALE = D ** -0.5

K_REP = int(os.environ.get("BASS_K_REP", "1"))

_f32 = mybir.dt.float32
_BF = mybir.dt.bfloat16
_NP_BF = mybir.dt.np(_BF)


def preprocess(x, rope, qkv_w, q_bias, v_bias, proj_w, proj_b):
    """Host-side prep: transposes, head-d permutation, rope tables."""
    perm = np.concatenate([np.arange(0, D, 2), np.arange(1, D, 2)])
    rows = np.arange(3 * C)
    for region in (0, 1):  # q, k head-d reorder; v untouched
        for h in range(H):
            base = region * C + h * D
            rows[base : base + D] = base + perm
    qkv_w_p = np.asarray(qkv_w, np.float32)[rows]
    qkvb_flat = np.concatenate(
        [np.asarray(q_bias, np.float32), np.zeros(C, np.float32),
         np.asarray(v_bias, np.float32)])[rows]
    # [128, 18] column-per-o-tile layout for per-partition ACT bias
    qkvb = np.ascontiguousarray(qkvb_flat.reshape(18, 128).T)

    wT = np.ascontiguousarray(qkv_w_p.T).astype(_NP_BF)          # [768, 2304]
    pwT = np.ascontiguousarray(np.asarray(proj_w, np.float32).T).astype(_NP_BF)

    rope = np.asarray(rope, np.float32)
    sinT = np.ascontiguousarray(rope[:, :D].T[perm])             # [64, 576]
    cosT = np.ascontiguousarray(rope[:, D:].T[perm])
    cosT2 = np.concatenate([cosT, cosT], 0).astype(_NP_BF)       # [128, 576]
    sinT2 = np.concatenate([sinT, sinT], 0).astype(_NP_BF)

    rotm = np.zeros((128, 128), np.float32)
    for blk in range(2):
        o = blk * 64
        for j in range(32):
            rotm[o + 32 + j, o + j] = -1.0   # out[j]    = -rhs[32+j]
            rotm[o + j, o + 32 + j] = 1.0    # out[32+j] = +rhs[j]
    rotm = rotm.astype(_NP_BF)

    x = np.asarray(x, np.float32)
    xTs = []
    for core in range(N_CORES):
        xc = x[core * NB : (core + 1) * NB]                      # [NB, 577, 768]
        xTs.append(np.ascontiguousarray(
            xc.transpose(2, 0, 1).reshape(C, TT)).astype(_NP_BF))

    vb = np.asarray(v_bias, np.float32)
    pb = np.asarray(proj_b, np.float32)
    zero_bias = (not qkvb_flat.any()) and (not pb.any())
    return xTs, dict(wT=wT, pwT=pwT, qkvb=qkvb, vb=vb, pb=pb,
                     cosT2=cosT2, sinT2=sinT2, rotm=rotm), zero_bias


def build(zero_bias=True, k_rep=K_REP):
    nc = bacc.Bacc("TRN2", target_bir_lowering=False, debug=False,
                   num_devices=N_CORES)

    d_xT = nc.dram_tensor("xT", [C, TT], _BF, kind="ExternalInput").ap()
    d_wT = nc.dram_tensor("wT", [C, 3 * C], _BF, kind="ExternalInput").ap()
    d_pwT = nc.dram_tensor("pwT", [C, C], _BF, kind="ExternalInput").ap()
    d_qkvb = nc.dram_tensor("qkvb", [128, 18], _f32, kind="ExternalInput").ap()
    d_vb = nc.dram_tensor("vb", [C], _f32, kind="ExternalInput").ap()
    d_pb = nc.dram_tensor("pb", [C], _f32, kind="ExternalInput").ap()
    d_cos = nc.dram_tensor("cosT2", [128, N - 1], _BF, kind="ExternalInput").ap()
    d_sin = nc.dram_tensor("sinT2", [128, N - 1], _BF, kind="ExternalInput").ap()
    d_rotm = nc.dram_tensor("rotm", [128, 128], _BF, kind="ExternalInput").ap()
    d_out = nc.dram_tensor("out", [TT, C], _f32, kind="ExternalOutput").ap()

    Id = mybir.ActivationFunctionType.Identity
    Exp = mybir.ActivationFunctionType.Exp

    def _row_bc(ap, parts):
        return bass_mod.AP(tensor=ap.tensor, offset=ap.offset,
                           ap=[[0, parts]] + list(ap.ap))

    with tile.TileContext(nc) as tc:
        with tc.tile_pool(name="main", bufs=1) as pool, \
             tc.tile_pool(name="ps", bufs=1, space="PSUM") as pspool, \
             tc.tile_pool(name="dr", bufs=1, space="DRAM") as drpool:

            # ---- resident constants ----
            wT_sb = pool.tile([128, 6, 3 * C], _BF, tag="wT")
            pwT_sb = pool.tile([128, 6, C], _BF, tag="pwT")
            for c in range(6):
                nc.sync.dma_start(out=wT_sb[:, c, :], in_=d_wT[c * 128:(c + 1) * 128, :])
                nc.sync.dma_start(out=pwT_sb[:, c, :], in_=d_pwT[c * 128:(c + 1) * 128, :])
            cos_sb = pool.tile([128, N - 1], _BF, tag="cos")
            nc.sync.dma_start(out=cos_sb, in_=d_cos)
            sin_sb = pool.tile([128, N - 1], _BF, tag="sin")
            nc.sync.dma_start(out=sin_sb, in_=d_sin)
            rotm_sb = pool.tile([128, 128], _BF, tag="rotm")
            nc.sync.dma_start(out=rotm_sb, in_=d_rotm)

            if not zero_bias:
                qkvb_sb = pool.tile([128, 18], _f32, tag="qkvb")
                nc.sync.dma_start(out=qkvb_sb, in_=d_qkvb)
                vb_bc = pool.tile([128, C], _f32, tag="vb")
                nc.sync.dma_start(out=vb_bc, in_=_row_bc(d_vb, 128))
                pb_bc = pool.tile([128, C], _f32, tag="pb")
                nc.sync.dma_start(out=pb_bc, in_=_row_bc(d_pb, 128))

            # resident xT (whole core's tokens; col TT..TT+1 zero pad)
            xT_sb = pool.tile([128, 6, TT + 2], _BF, tag="xT")
            nc.vector.memset(xT_sb[:, :, TT:TT + 2], 0.0)
            for c in range(6):
                nc.sync.dma_start(out=xT_sb[:, c, 0:TT],
                                  in_=d_xT[c * 128:(c + 1) * 128, :])

            def body():
                for b in range(NB):
                    t0 = b * N
                    xs = lambda c: xT_sb[:, c, t0:t0 + N2]

                    qk_sb = pool.tile([128, 12, N2], _BF, tag="qk", bufs=2)
                    v_sb = pool.tile([128, 5, H, D + 2], _BF, tag="vsb", bufs=2)
                    oT_sb = pool.tile([128, 6, N2], _BF, tag="oT", bufs=2)
                    nc.vector.memset(qk_sb[:, :, N:N2], 0.0)
                    nc.vector.memset(oT_sb[:, :, N:N2], 0.0)
                    # ones columns everywhere, then zero the pad rows
                    # (65:128) of the last k-chunk, then restore row 64's
                    # ones (row 64 = token 576's V, rewritten by phase B;
                    # rows 65: must be fully zero to drop out of PV).
                    nc.vector.memset(v_sb[:, :, :, D:D + 2], 1.0)
                    nc.vector.memset(v_sb[64:128, 4, :, :], 0.0)
                    nc.vector.memset(v_sb[64:65, 4, :, D:D + 2], 1.0)

                    # ---- A: q,k (o-major) ----
                    for ot in range(12):
                        psA = pspool.tile([128, 512], _f32, tag="w512", bufs=5)
                        psA2 = pspool.tile([128, 66], _f32, tag="w66", bufs=3)
                        for c in range(6):
                            lhsT = wT_sb[:, c, ot * 128:(ot + 1) * 128]
                            nc.tensor.matmul(psA, lhsT, xs(c)[:, 0:512],
                                             start=c == 0, stop=c == 5)
                            nc.tensor.matmul(psA2, lhsT, xs(c)[:, 512:N2],
                                             start=c == 0, stop=c == 5)
                        if zero_bias:
                            nc.scalar.activation(qk_sb[:, ot, 0:512], psA, Id)
                            nc.scalar.activation(qk_sb[:, ot, 512:N2],
                                                 psA2, Id)
                        else:
                            bias = qkvb_sb[:, ot:ot + 1]
                            nc.scalar.activation(qk_sb[:, ot, 0:512], psA, Id,
                                                 bias=bias)
                            nc.scalar.activation(qk_sb[:, ot, 512:N],
                                                 psA2[:, 0:65], Id, bias=bias)

                    # ---- B: v (t-major) ----
                    for tt in range(5):
                        P = min(128, N - tt * 128)   # true rows (65 on last)
                        PM = P + (P % 2)
                        psB = pspool.tile([128, 512], _f32, tag="w512", bufs=5)
                        psB2 = pspool.tile([128, 256], _f32, tag="w66", bufs=3)
                        for c in range(6):
                            lhsT = xs(c)[:, tt * 128:tt * 128 + PM]
                            nc.tensor.matmul(psB[:PM], lhsT,
                                             wT_sb[:, c, 1536:2048],
                                             start=c == 0, stop=c == 5)
                            nc.tensor.matmul(psB2[:PM], lhsT,
                                             wT_sb[:, c, 2048:2304],
                                             start=c == 0, stop=c == 5)
                        if zero_bias:
                            nc.vector.tensor_copy(
                                v_sb[0:P, tt, 0:8, 0:D],
                                psB[:P].rearrange("p (h d) -> p h d", d=D))
                            nc.vector.tensor_copy(
                                v_sb[0:P, tt, 8:12, 0:D],
                                psB2[:P].rearrange("p (h d) -> p h d", d=D))
                        else:
                            nc.vector.tensor_add(
                                v_sb[0:P, tt, 0:8, 0:D],
                                psB[:P].rearrange("p (h d) -> p h d", d=D),
                                vb_bc[0:P, 0:512].rearrange("p (h d) -> p h d", d=D))
                            nc.vector.tensor_add(
                                v_sb[0:P, tt, 8:12, 0:D],
                                psB2[:P].rearrange("p (h d) -> p h d", d=D),
                                vb_bc[0:P, 512:768].rearrange("p (h d) -> p h d", d=D))

                    # ---- R: rope on q,k ----
                    for ot in range(12):
                        psR = pspool.tile([128, 512], _f32, tag="w512", bufs=5)
                        psR2 = pspool.tile([128, 64], _f32, tag="w66", bufs=3)
                        nc.tensor.matmul(psR, rotm_sb, qk_sb[:, ot, 1:513],
                                         start=True, stop=True)
                        nc.tensor.matmul(psR2, rotm_sb,
                                         qk_sb[:, ot, 513:N],
                                         start=True, stop=True)
                        tmp = pool.tile([128, N - 1], _BF, tag="rtmp", bufs=2)
                        nc.vector.tensor_mul(tmp[:, 0:512], psR, sin_sb[:, 0:512])
                        nc.vector.tensor_mul(tmp[:, 512:N - 1], psR2,
                                             sin_sb[:, 512:N - 1])
                        nc.vector.tensor_mul(qk_sb[:, ot, 1:N], qk_sb[:, ot, 1:N],
                                             cos_sb)
                        nc.vector.tensor_add(qk_sb[:, ot, 1:N], qk_sb[:, ot, 1:N],
                                             tmp)

                    # ---- attention per head pair ----
                    for ot in range(6):
                        E = [pool.tile([128, 5, N2], _BF, tag="E", bufs=4,
                                       name=f"E{hb}")
                             for hb in (0, 64)]
                        psS66 = [pspool.tile([128, 5, 66], _f32, tag="w66",
                                             bufs=3, name=f"psS66_{hb}")
                                 for hb in (0, 64)]
                        # S + exp, pair-concurrent on PE
                        for kc in range(5):
                            KP = min(128, N2 - kc * 128)  # 128 or 66 padded
                            psS = [pspool.tile([128, 512], _f32, tag="w512",
                                               bufs=5, name=f"psS_{hb}")
                                   for hb in (0, 64)]
                            for i, hb in enumerate((0, 64)):
                                kk = qk_sb[hb:hb + 64, 6 + ot,
                                           kc * 128:kc * 128 + KP]
                                qq1 = qk_sb[hb:hb + 64, ot, 0:512]
                                nc.tensor.matmul(psS[i][:KP], kk, qq1,
                                                 start=True, stop=True)
                            for i, hb in enumerate((0, 64)):
                                kk = qk_sb[hb:hb + 64, 6 + ot,
                                           kc * 128:kc * 128 + KP]
                                qq2 = qk_sb[hb:hb + 64, ot, 512:N2]
                                nc.tensor.matmul(psS66[i][:KP, kc, :], kk, qq2,
                                                 start=True, stop=True)
                            for i in range(2):
                                nc.scalar.activation(E[i][0:KP, kc, 0:512],
                                                     psS[i][:KP], Exp,
                                                     scale=SCALE)
                        # merged tail exps (rows beyond KP of kc=4 are junk
                        # PSUM; exp only the valid rows)
                        for i in range(2):
                            nc.scalar.activation(E[i][:, 0:4, 512:N2],
                                                 psS66[i][:, 0:4, :], Exp,
                                                 scale=SCALE)
                            nc.scalar.activation(E[i][0:66, 4, 512:N2],
                                                 psS66[i][0:66, 4, :], Exp,
                                                 scale=SCALE)

                        # PV accumulation: 512 cols in w512, 66-tails packed
                        psO = [pspool.tile([128, 512], _f32, tag="w512",
                                           bufs=5, name=f"psO_{hb}")
                               for hb in (0, 64)]
                        # per-head tail tiles: start=True clears has_written
                        # for the WHOLE bank, so accumulating groups must not
                        # share a bank
                        psO66 = [pspool.tile([128, 66], _f32, tag="w66",
                                             bufs=3, name=f"psO66_{hb}")
                                 for hb in (0, 64)]
                        for kc in range(5):
                            KP = min(128, N2 - kc * 128)
                            for i, hb in enumerate((0, 64)):
                                vv = v_sb[0:KP, kc, 2 * ot + i, :]
                                nc.tensor.matmul(psO[i][:D + 2, :], vv,
                                                 E[i][0:KP, kc, 0:512],
                                                 start=kc == 0, stop=kc == 4)
                                nc.tensor.matmul(psO66[i][:D + 2, :], vv,
                                                 E[i][0:KP, kc, 512:N2],
                                                 start=kc == 0, stop=kc == 4)

                        # renorm: reciprocal -> DRAM-staged broadcast DMA ->
                        # DVE mul (keeps PE out of the per-head dep chain)
                        for i, hb in enumerate((0, 64)):
                            r2 = pool.tile([66, N2], _f32, tag="r2", bufs=4)
                            nc.vector.reciprocal(r2[64:65, 0:512],
                                                 psO[i][64:65, :])
                            nc.vector.reciprocal(r2[64:65, 512:N2],
                                                 psO66[i][64:65, :])
                            rrow = drpool.tile([1, N2], _f32, tag="rrow", bufs=8)
                            nc.sync.dma_start(out=rrow, in_=r2[64:65, :])
                            rbc = pool.tile([64, N2], _f32, tag="rbc", bufs=4)
                            nc.sync.dma_start(out=rbc, in_=_row_bc(rrow[0, :], 64))
                            if hb == 0:
                                nc.vector.tensor_mul(oT_sb[0:64, ot, 0:512],
                                                     psO[i][0:64, :],
                                                     rbc[:, 0:512])
                                nc.vector.tensor_mul(oT_sb[0:64, ot, 512:N2],
                                                     psO66[i][0:64, :],
                                                     rbc[:, 512:N2])
                            else:
                                otmp = pool.tile([64, N2], _BF, tag="otmp",
                                                 bufs=3)
                                nc.vector.tensor_mul(otmp[:, 0:512],
                                                     psO[i][0:64, :],
                                                     rbc[:, 0:512])
                                nc.vector.tensor_mul(otmp[:, 512:N2],
                                                     psO66[i][0:64, :],
                                                     rbc[:, 512:N2])
                                nc.sync.dma_start(out=oT_sb[64:128, ot, 0:N2],
                                                  in_=otmp)

                    # ---- P: output projection ----
                    for tt in range(5):
                        P = min(128, N - tt * 128)
                        PM = P + (P % 2)
                        psP = pspool.tile([128, 512], _f32, tag="w512", bufs=5)
                        psP2 = pspool.tile([128, 256], _f32, tag="w66", bufs=3)
                        for c in range(6):
                            lhsT = oT_sb[:, c, tt * 128:tt * 128 + PM]
                            nc.tensor.matmul(psP[:PM], lhsT, pwT_sb[:, c, 0:512],
                                             start=c == 0, stop=c == 5)
                            nc.tensor.matmul(psP2[:PM], lhsT,
                                             pwT_sb[:, c, 512:C],
                                             start=c == 0, stop=c == 5)
                        yt = pool.tile([128, C], _f32, tag="y", bufs=2)
                        if zero_bias:
                            nc.vector.tensor_copy(yt[:P, 0:512], psP[:P])
                            nc.vector.tensor_copy(yt[:P, 512:C], psP2[:P])
                        else:
                            nc.vector.tensor_add(yt[:P, 0:512], psP[:P],
                                                 pb_bc[:P, 0:512])
                            nc.vector.tensor_add(yt[:P, 512:C], psP2[:P],
                                                 pb_bc[:P, 512:C])
                        nc.sync.dma_start(
                            out=d_out[t0 + tt * 128:t0 + tt * 128 + P, :],
                            in_=yt[:P, :])

            if k_rep > 1:
                with tc.For_i(0, k_rep, 1):
                    body()
            else:
                body()

    nc.compile()
    return nc


_CACHE = {}


def _get_nc(zero_bias=True, k_rep=K_REP):
    key = (zero_bias, k_rep)
    if key not in _CACHE:
        _CACHE[key] = build(zero_bias, k_rep)
    return _CACHE[key]


def kernel(**inputs) -> np.ndarray:
    xTs, pre, zero_bias = preprocess(**inputs)
    nc = _get_nc(zero_bias)
    shared = {k: pre[k] for k in
              ("wT", "pwT", "qkvb", "vb", "pb", "cosT2", "sinT2", "rotm")}
    in_maps = [dict(shared, xT=xTs[core]) for core in range(N_CORES)]
    res = run_bass_kernel_spmd(nc, in_maps, list(range(N_CORES)))
    out = np.concatenate(
        [res.results[c]["out"].reshape(NB, N, C) for c in range(N_CORES)], axis=0)
    return out
